# revision 44
# baseline (speedup 1.0000x reference)
"""Submanifold 3x3x3 sparse conv (gnn_message_passing) + BatchNorm + LeakyReLU
on 8 Trainium2 NeuronCores.

Strategy (N=200000, C=128, K=27, GRID=128^3 @ ~9.5% occupancy):
  * Whole-component LPT partition across the 8 cores: every neighbor
    reference stays inside its core's shard; shard-local indices fit in
    int16 for the SWDGE gather/scatter ucode.
  * At 9.5% occupancy only ~2.44 of the 26 non-self neighbor slots are
    valid, and the SWDGE path runs at a fixed per-token rate regardless of
    chunking, so this kernel only pays for valid pairs (vs 26*N for the
    dense formulation):
      - host builds, per offset k != 13, the compact list of valid
        (dst_row, src_row) pairs, padded to a shared per-k budget
        (multiple of 128, max across cores, so the SPMD program is static);
      - gather (SWDGE queue 0): SBUF-source dma_gather of src rows ->
        G [128ci, n_k] bf16 (pads fetch the zero row);
      - matmul per 128-token block: Z[tok, co] = G_blk^T @ W[k] into PSUM
        (4 blocks per PSUM tile, one ACT copy to the SBUF stage);
      - dma_scatter_add f32 (SWDGE queue 1 - separate queue so the two
        SWDGE streams overlap) into one of 3 DRAM accumulators, assigned
        by k-group. Within one k the dst rows are unique so descriptors
        never race; same-buffer instructions are ordered by the tile
        framework + queue FIFO.
  * The self offset (k=13) is dense: per supertile, PE-transpose the
    table rows, Z_self^T = W[13]^T @ X^T, transpose back, and write as
    accumulator 0's initializer (BatchNorm cancels the conv bias b).
    Buffers 1, 2 are zero-filled.
  * k-group buffers let the tail start early: buffer g is complete once
    group g's scatters drain, so its readback + combine into bf16 SBUF
    tiles overlaps the remaining groups' gather/scatter stream. Only the
    last group's readback + stats (per-channel sum/sumsq via ones-matmul),
    the 1KB AllReduce, and the BN apply (DVE affine + ACT Prelu
    alpha=0.333) sit after the final scatter.

Falls back to a pure-numpy reference computation if the input graph is not
separable into <=25088-row shards (never the case for the intended input
distribution).
"""

import numpy as np
import ml_dtypes

C = 128
K = 27
EPS = 1e-4
LEAK = 0.333
N_CORES = 8
SELF_K = 13
ST = 512                 # supertile rows (self/tail phases)
SHARD = 25_088           # padded rows per core (49 supertiles)
N_ST = SHARD // ST
TABLE_ROWS = 25_216      # shard table rows incl. zero pad
ZERO_ROW = TABLE_ROWS - 1
DUMP_ROW = TABLE_ROWS - 1   # scatter pad target (beyond SHARD)
N_TOTAL = 200_000
KG_LIST = [k for k in range(K) if k != SELF_K]
GCHUNK = 512             # max idxs per single-packet SWDGE instruction


class Plan:
    """Static per-build parameters shared by all cores (SPMD program)."""

    def __init__(self, budgets):
        assert len(budgets) == len(KG_LIST)
        self.budgets = budgets            # per-k token budget, mult of 128
        self.n_cores = N_CORES
        self.n_total = N_TOTAL
        self.nbuf = 3
        self.nqueues = 2
        self.gather_queues = [0]
        self.scatter_queues = [1]

    @property
    def total_tokens(self):
        return sum(self.budgets)

    def chunks(self, k_i):
        b = self.budgets[k_i]
        out = [GCHUNK] * (b // GCHUNK)
        if b % GCHUNK:
            out.append(b % GCHUNK)
        return out

    def group_of(self, ki):
        nkg = len(KG_LIST)
        return min(ki * self.nbuf // nkg, self.nbuf - 1)


def _partition_components(nb, n, n_cores, shard_cap):
    """Whole-component LPT partition. Returns (members_per_core, ok)."""
    import scipy.sparse as sp
    import scipy.sparse.csgraph as csg
    import heapq

    valid = nb >= 0
    ii, kk = np.nonzero(valid)
    jj = nb[ii, kk]
    m = kk != SELF_K
    g = sp.coo_matrix((np.ones(m.sum(), np.int8), (ii[m], jj[m])), shape=(n, n))
    _, labels = csg.connected_components(g, directed=False)
    sizes = np.bincount(labels)
    if sizes.max() > shard_cap:
        return None, False
    order = np.argsort(sizes)[::-1]
    heap = [(0, c) for c in range(n_cores)]
    heapq.heapify(heap)
    assign = np.empty(len(sizes), np.int32)
    for comp in order:
        load, c = heapq.heappop(heap)
        assign[comp] = c
        heapq.heappush(heap, (load + int(sizes[comp]), c))
    if max(l for l, _ in heap) > shard_cap:
        return None, False
    shard_of = assign[labels]
    members = [np.nonzero(shard_of == c)[0] for c in range(n_cores)]
    return members, True


def _wrap_idx(idx16):
    """k-major token list [n] int16 -> [128, n/16] wrapped + replicated."""
    n = idx16.shape[0]
    assert n % 16 == 0
    wrapped = idx16.reshape(n // 16, 16).T          # [16, cols]
    return np.ascontiguousarray(np.tile(wrapped, (8, 1)))


def _make_plan(nb, members):
    """Per-k budgets = max over cores of the valid-pair count, rounded up."""
    budgets = []
    counts = np.zeros((len(members), len(KG_LIST)), np.int64)
    for ci, mem in enumerate(members):
        nbm = nb[mem]
        for ki, k in enumerate(KG_LIST):
            counts[ci, ki] = int((nbm[:, k] >= 0).sum())
    for ki in range(len(KG_LIST)):
        b = int(counts[:, ki].max())
        b = max(128, ((b + 127) // 128) * 128)
        budgets.append(b)
    return Plan(budgets)


def _prepare_core_inputs(features, nb, members, plan):
    n = features.shape[0]
    loc = np.full(n, ZERO_ROW, np.int32)
    for mem in members:
        loc[mem] = np.arange(len(mem), dtype=np.int32)

    in_maps = []
    for mem in members:
        real = len(mem)
        assert real <= SHARD
        table = np.zeros((TABLE_ROWS, C), ml_dtypes.bfloat16)
        table[:real] = features[mem].astype(ml_dtypes.bfloat16)

        nb_loc = nb[mem]                                  # [real, K]
        g_list, s_list = [], []
        for ki, k in enumerate(KG_LIST):
            col = nb_loc[:, k]
            ii = np.nonzero(col >= 0)[0]                  # dst rows (unique)
            jj = loc[col[ii]]                             # src rows, local
            assert (jj < real).all(), "neighbor escaped shard"
            b = plan.budgets[ki]
            assert len(ii) <= b, (len(ii), b)
            gpad = np.full(b, ZERO_ROW, np.int32)
            spad = np.full(b, DUMP_ROW, np.int32)
            gpad[:len(jj)] = jj
            spad[:len(ii)] = ii
            g_list.append(gpad)
            s_list.append(spad)
        g_idx = np.concatenate(g_list).astype(np.int16)
        s_idx = np.concatenate(s_list).astype(np.int16)
        in_maps.append({
            "table": table,
            "gidx": _wrap_idx(g_idx),
            "sidx": _wrap_idx(s_idx),
        })
    return in_maps


def emit_kernel(tc, out_ap, ins, plan):
    import concourse.mybir as mybir
    from concourse.bass import ts
    from concourse.masks import make_identity

    nc = tc.nc
    F32 = mybir.dt.float32
    BF16 = mybir.dt.bfloat16
    I16 = mybir.dt.int16
    NKG = len(KG_LIST)
    NB = ST // 128
    LOOKAHEAD = 4
    NBUF = plan.nbuf

    table = ins["table"]
    gidx, sidx, w = ins["gidx"], ins["sidx"], ins["w"]
    gamma, beta = ins["gamma"], ins["beta"]
    idx_cols = plan.total_tokens // 16
    koff = np.cumsum([0] + plan.budgets).tolist()
    # last ki of each k-group (buffer g completes after this ki's scatters)
    group_last = {}
    for ki in range(NKG):
        group_last[plan.group_of(ki)] = ki

    with (
        tc.tile_pool(name="const", bufs=1) as constp,
        tc.tile_pool(name="dram", bufs=1, space="DRAM") as dramp,
    ):
        # ---- constants ----
        table_sb = constp.tile([128, (TABLE_ROWS // 128) * C], BF16)
        nc.sync.dma_start(
            table_sb[:].rearrange("p (r c) -> p r c", r=TABLE_ROWS // 128),
            table.rearrange("(r p) c -> p r c", p=128))
        w_sb = constp.tile([128, K * C], BF16)
        nc.sync.dma_start(w_sb[:].rearrange("ci (k co) -> ci k co", k=K),
                          w.rearrange("k ci co -> ci k co"))
        gidx_sb = constp.tile([128, idx_cols], I16)
        nc.sync.dma_start(gidx_sb[:], gidx)
        sidx_sb = constp.tile([128, idx_cols], I16)
        nc.sync.dma_start(sidx_sb[:], sidx)
        gamma_sb = constp.tile([1, C], F32)
        nc.sync.dma_start(gamma_sb[:], gamma[None, :])
        beta_sb = constp.tile([1, C], F32)
        nc.sync.dma_start(beta_sb[:], beta[None, :])
        identity = constp.tile([128, 128], F32)
        make_identity(nc, identity[:])
        identity_bf = constp.tile([128, 128], BF16)
        nc.vector.tensor_copy(identity_bf[:], identity[:])
        ones_sb = constp.tile([128, 1], BF16)
        nc.vector.memset(ones_sb[:], 1.0)
        zero128 = constp.tile([128, 1, C], F32)
        nc.vector.memset(zero128[:], 0.0)
        zero_st = constp.tile([128, NB, C], F32)
        nc.vector.memset(zero_st[:], 0.0)

        pres = [dramp.tile([TABLE_ROWS, C], F32, name=f"pre{i}")
                for i in range(NBUF)]

        # ---- self offset: pres[0][r] = W[13]^T x[r]; zero pres[1:] ----
        with (
            tc.tile_pool(name="selfw", bufs=3) as selfw,
            tc.tile_pool(name="selfps", bufs=2, space="PSUM") as selfps,
        ):
            for s in range(N_ST):
                xtp = selfps.tile([128, ST], BF16)
                for b in range(NB):
                    nc.tensor.transpose(xtp[:, ts(b, 128)],
                                        table_sb[:, ts(NB * s + b, C)],
                                        identity_bf[:])
                xcol = selfw.tile([128, ST], BF16)
                nc.vector.tensor_copy(xcol[:], xtp[:])
                ps = selfps.tile([128, ST], F32)
                nc.tensor.matmul(ps[:], w_sb[:, ts(SELF_K, C)], xcol[:],
                                 start=True, stop=True)
                zs = selfw.tile([128, ST], F32)
                nc.scalar.copy(zs[:], ps[:])
                pt = selfps.tile([128, ST], F32)
                for b in range(NB):
                    nc.tensor.transpose(pt[:, ts(b, 128)], zs[:, ts(b, 128)],
                                        identity[:])
                stage = selfw.tile([128, ST], F32)
                nc.vector.tensor_copy(stage[:], pt[:])
                nc.sync.dma_start(
                    pres[0][s * ST:(s + 1) * ST, :].rearrange(
                        "(b p) c -> p b c", p=128),
                    stage[:].rearrange("p (b c) -> p b c", b=NB))
                for pz in pres[1:]:
                    nc.sync.dma_start(
                        pz[s * ST:(s + 1) * ST, :].rearrange(
                            "(b p) c -> p b c", p=128),
                        zero_st[:])
            for pz in pres:
                nc.sync.dma_start(
                    pz[SHARD:TABLE_ROWS, :].rearrange("(b p) c -> p b c",
                                                      p=128),
                    zero128[:])

        # ---- main: per-offset gather -> matmul -> scatter-add;
        #      per-group early readback + combine into bf16 SBUF tiles ----
        xbpool = tc.alloc_tile_pool(name="xbpool", bufs=N_ST)
        xbtiles = [None] * N_ST
        statps = tc.alloc_tile_pool(name="statps", bufs=1, space="PSUM")
        sum_ps = statps.tile([1, ST], F32)
        sq_ps = statps.tile([1, ST], F32)

        with (
            tc.tile_pool(name="gath", bufs=LOOKAHEAD + 1) as gathp,
            tc.tile_pool(name="zst", bufs=3) as zstp,
            tc.tile_pool(name="psum", bufs=4, space="PSUM") as psump,
            tc.tile_pool(name="statw", bufs=4) as statw,
        ):
            gprobe = None
            if getattr(plan, "skip_gather", False):  # perf probe only
                gprobe = constp.tile([128, 1, max(plan.budgets)], BF16)
                nc.vector.memset(gprobe[:], 0)

            gqueues = plan.gather_queues
            gq_count = [0]

            def do_gather(ki):
                if gprobe is not None:  # perf probe only
                    return gprobe
                b = plan.budgets[ki]
                g = gathp.tile([128, 1, b], BF16)
                off = koff[ki]
                pos = 0
                for ch in plan.chunks(ki):
                    q = gqueues[gq_count[0] % len(gqueues)]
                    gq_count[0] += 1
                    nc.gpsimd.dma_gather(
                        g[:, :, pos:pos + ch], table_sb[:],
                        gidx_sb[:, (off + pos) // 16:(off + pos + ch) // 16],
                        ch, ch, C,
                        transpose=True, single_packet=True,
                        sbuf_tokens_per_rank=128,
                        sbuf_free_dim_per_rank=C * 2,
                        queue_num=q)
                    pos += ch
                return g

            def readback_group(grp):
                """Buffer `grp` is complete: read it back and fold into the
                bf16 combine tiles (last group also feeds the stats)."""
                last = (grp == NBUF - 1)
                pz = pres[grp]
                for s in range(N_ST):
                    xt = statw.tile([128, NB, C], F32)
                    nc.sync.dma_start(
                        xt[:], pz[s * ST:(s + 1) * ST, :].rearrange(
                            "(b p) c -> p b c", p=128))
                    if grp == 0:
                        xb = xbpool.tile([128, NB * C], BF16)
                        nc.vector.tensor_copy(
                            xb[:], xt[:].rearrange("p b c -> p (b c)"))
                        xbtiles[s] = xb
                    else:
                        xb = xbtiles[s]
                        nc.vector.tensor_tensor(
                            out=xb[:], in0=xb[:],
                            in1=xt[:].rearrange("p b c -> p (b c)"),
                            op=mybir.AluOpType.add)
                    if last:
                        sq = statw.tile([128, NB * C], BF16)
                        nc.scalar.activation(
                            sq[:], xb[:], mybir.ActivationFunctionType.Square)
                        nc.tensor.matmul(sum_ps[:], ones_sb[:], xb[:],
                                         start=(s == 0), stop=(s == N_ST - 1))
                        nc.tensor.matmul(sq_ps[:], ones_sb[:], sq[:],
                                         start=(s == 0), stop=(s == N_ST - 1))

            gtiles = {}
            sc_count = [0]
            for ki in range(min(LOOKAHEAD, NKG)):
                gtiles[ki] = do_gather(ki)

            for ki, k in enumerate(KG_LIST):
                b = plan.budgets[ki]
                nblk = b // 128
                g = gtiles.pop(ki)
                zst = zstp.tile([128, nblk, C], F32)
                for g0 in range(0, nblk, 4):
                    g1 = min(g0 + 4, nblk)
                    zp = psump.tile([128, 4, C], F32)
                    for blk in range(g0, g1):
                        nc.tensor.matmul(zp[:, blk - g0, :],
                                         g[:, 0, ts(blk, 128)],
                                         w_sb[:, ts(k, C)],
                                         start=True, stop=True)
                    nc.scalar.copy(zst[:, g0:g1, :], zp[:, 0:g1 - g0, :])
                off = koff[ki]
                pos = 0
                dst = pres[plan.group_of(ki)]
                squeues = plan.scatter_queues
                for ch in plan.chunks(ki):
                    if getattr(plan, "skip_scatter", False):  # perf probe
                        break
                    q = squeues[sc_count[0] % len(squeues)]
                    sc_count[0] += 1
                    nc.gpsimd.dma_scatter_add(
                        dst[:, :], zst[:, pos // 128:(pos + ch) // 128, :],
                        sidx_sb[:, (off + pos) // 16:(off + pos + ch) // 16],
                        ch, ch, C, single_packet=True, queue_num=q)
                    pos += ch
                if ki + LOOKAHEAD < NKG:
                    gtiles[ki + LOOKAHEAD] = do_gather(ki + LOOKAHEAD)
                for grp, lki in group_last.items():
                    if lki == ki:
                        readback_group(grp)

        # ---- stats finalize -> allreduce -> BN coefficients ----
        stats_sb = constp.tile([1, 2 * C], F32)
        acc = constp.tile([1, 2 * C], F32)
        nc.vector.tensor_copy(acc[:, 0:C], sum_ps[:, 0:C])
        nc.vector.tensor_copy(acc[:, C:2 * C], sq_ps[:, 0:C])
        for b in range(1, NB):
            nc.vector.tensor_tensor(out=acc[:, 0:C], in0=acc[:, 0:C],
                                    in1=sum_ps[:, ts(b, C)],
                                    op=mybir.AluOpType.add)
            nc.vector.tensor_tensor(out=acc[:, C:2 * C], in0=acc[:, C:2 * C],
                                    in1=sq_ps[:, ts(b, C)],
                                    op=mybir.AluOpType.add)
        nc.vector.tensor_copy(stats_sb[:], acc[:])
        statps.release()

        if plan.n_cores > 1 and not getattr(plan, "skip_collective", False):
            stats_in = dramp.tile([1, 2 * C], F32)
            stats_out = dramp.tile([1, 2 * C], F32)
            nc.sync.dma_start(stats_in[:], stats_sb[:])
            nc.gpsimd.collective_compute(
                "AllReduce", mybir.AluOpType.add,
                replica_groups=[list(range(plan.n_cores))],
                ins=[stats_in.opt()], outs=[stats_out.opt()],
            )
            stats2 = constp.tile([1, 2 * C], F32)
            nc.sync.dma_start(stats2[:], stats_out[:])
        else:
            stats2 = stats_sb

        mean_t = constp.tile([1, C], F32)
        ex2_t = constp.tile([1, C], F32)
        var_t = constp.tile([1, C], F32)
        std_t = constp.tile([1, C], F32)
        rstd_t = constp.tile([1, C], F32)
        s_vec = constp.tile([1, C], F32)
        t_vec = constp.tile([1, C], F32)
        tmp = constp.tile([1, C], F32)
        inv_n = 1.0 / plan.n_total
        nc.vector.tensor_scalar_mul(mean_t[:], stats2[:, 0:C], inv_n)
        nc.vector.tensor_scalar_mul(ex2_t[:], stats2[:, C:2 * C], inv_n)
        nc.vector.tensor_tensor(out=tmp[:], in0=mean_t[:], in1=mean_t[:],
                                op=mybir.AluOpType.mult)
        nc.vector.tensor_tensor(out=var_t[:], in0=ex2_t[:], in1=tmp[:],
                                op=mybir.AluOpType.subtract)
        nc.vector.tensor_scalar_add(var_t[:], var_t[:], EPS)
        nc.scalar.activation(std_t[:], var_t[:],
                             mybir.ActivationFunctionType.Sqrt)
        nc.vector.reciprocal(rstd_t[:], std_t[:])
        nc.vector.tensor_tensor(out=s_vec[:], in0=rstd_t[:], in1=gamma_sb[:],
                                op=mybir.AluOpType.mult)
        nc.vector.tensor_tensor(out=tmp[:], in0=mean_t[:], in1=s_vec[:],
                                op=mybir.AluOpType.mult)
        nc.vector.tensor_tensor(out=t_vec[:], in0=beta_sb[:], in1=tmp[:],
                                op=mybir.AluOpType.subtract)

        s_bc = constp.tile([128, NB, C], F32)
        t_bc = constp.tile([128, NB, C], F32)
        nc.gpsimd.partition_broadcast(s_bc[:, 0, :], s_vec[:])
        nc.gpsimd.partition_broadcast(t_bc[:, 0, :], t_vec[:])
        for b in range(1, NB):
            nc.vector.tensor_copy(s_bc[:, b, :], s_bc[:, 0, :])
            nc.vector.tensor_copy(t_bc[:, b, :], t_bc[:, 0, :])

        # ---- BN apply + LeakyReLU + writeback ----
        with tc.tile_pool(name="applyw", bufs=3) as applyp:
            for s in range(N_ST):
                xb = xbtiles[s]
                y = applyp.tile([128, NB, C], F32)
                nc.vector.tensor_tensor(
                    out=y[:].rearrange("p b c -> p (b c)"), in0=xb[:],
                    in1=s_bc[:].rearrange("p b c -> p (b c)"),
                    op=mybir.AluOpType.mult)
                nc.vector.tensor_tensor(out=y[:], in0=y[:], in1=t_bc[:],
                                        op=mybir.AluOpType.add)
                yo = applyp.tile([128, NB, C], F32)
                nc.scalar.activation(yo[:].rearrange("p b c -> p (b c)"),
                                     y[:].rearrange("p b c -> p (b c)"),
                                     mybir.ActivationFunctionType.Prelu,
                                     alpha=LEAK)
                nc.sync.dma_start(
                    out_ap[s * ST:(s + 1) * ST, :].rearrange(
                        "(b p) c -> p b c", p=128),
                    yo[:])
        xbpool.release()


def _build_bass(plan, reps=1):
    import concourse.bacc as bacc
    import concourse.mybir as mybir
    import concourse.tile as tile

    nc = bacc.Bacc("TRN2", target_bir_lowering=False, debug=False,
                   num_devices=plan.n_cores,
                   num_swdge_queues=getattr(plan, "nqueues", 2))
    F32 = mybir.dt.float32
    BF16 = mybir.dt.bfloat16
    I16 = mybir.dt.int16
    idx_cols = plan.total_tokens // 16
    ins = {
        "table": nc.dram_tensor("table", [TABLE_ROWS, C], BF16,
                                kind="ExternalInput")[:, :],
        "gidx": nc.dram_tensor("gidx", [128, idx_cols], I16,
                               kind="ExternalInput")[:, :],
        "sidx": nc.dram_tensor("sidx", [128, idx_cols], I16,
                               kind="ExternalInput")[:, :],
        "w": nc.dram_tensor("w", [K, C, C], BF16, kind="ExternalInput")[:, :, :],
        "gamma": nc.dram_tensor("gamma", [C], F32, kind="ExternalInput")[:],
        "beta": nc.dram_tensor("beta", [C], F32, kind="ExternalInput")[:],
    }
    out = nc.dram_tensor("out", [SHARD, C], F32, kind="ExternalOutput")
    with tile.TileContext(nc) as tc:
        for _ in range(reps):
            emit_kernel(tc, out[:, :], ins, plan)
    nc.compile()
    return nc


def _reference_fallback(features, w, b, gamma, beta, nb):
    feats = np.asarray(features, np.float32)
    wf = np.asarray(w, np.float32)
    out = np.broadcast_to(np.asarray(b, np.float32), feats.shape).copy()
    valid = nb >= 0
    idx = np.where(valid, nb, 0)
    for k in range(K):
        xk = feats[idx[:, k]] * valid[:, k:k + 1]
        out += xk @ wf[k]
    mean = out.mean(0)
    var = out.var(0)
    out = (out - mean) / np.sqrt(var + EPS) * np.asarray(gamma, np.float32) \
        + np.asarray(beta, np.float32)
    return np.where(out > 0, out, LEAK * out).astype(np.float32)


def kernel(features, W, b, gamma, beta, neighbor_idx):
    from concourse.bass_utils import run_bass_kernel_spmd

    features = np.asarray(features, np.float32)
    Wf = np.asarray(W, np.float32)
    gamma_f = np.asarray(gamma, np.float32)
    beta_f = np.asarray(beta, np.float32)
    nb = np.asarray(neighbor_idx, np.int32)
    assert features.shape == (N_TOTAL, C)

    members, ok = _partition_components(nb, N_TOTAL, N_CORES, SHARD)
    if not ok:
        return _reference_fallback(features, Wf, b, gamma_f, beta_f, nb)

    plan = _make_plan(nb, members)
    core_maps = _prepare_core_inputs(features, nb, members, plan)
    w_bf = Wf.astype(ml_dtypes.bfloat16)
    for m in core_maps:
        m["w"] = w_bf
        m["gamma"] = gamma_f
        m["beta"] = beta_f

    nc = _build_bass(plan)
    res = run_bass_kernel_spmd(nc, core_maps, core_ids=list(range(N_CORES)))

    out_full = np.empty((N_TOTAL, C), np.float32)
    for c, mem in enumerate(members):
        out_full[mem] = res.results[c]["out"][:len(mem)]
    return out_full


# revision 49
# speedup vs baseline: 1.1366x; 1.1366x over previous
"""Submanifold 3x3x3 sparse conv (gnn_message_passing) + BatchNorm + LeakyReLU
on 8 Trainium2 NeuronCores.

Strategy (N=200000, C=128, K=27, GRID=128^3 @ ~9.5% occupancy):
  * Whole-component LPT partition across the 8 cores: every neighbor
    reference stays inside its core's shard; shard-local indices fit in
    int16 for the SWDGE gather/scatter ucode.
  * At 9.5% occupancy only ~2.44 of the 26 non-self neighbor slots are
    valid, and the SWDGE path runs at a fixed per-token rate regardless of
    chunking, so this kernel only pays for valid pairs (vs 26*N for the
    dense formulation):
      - host builds, per offset k != 13, the compact list of valid
        (dst_row, src_row) pairs, padded to a shared per-k budget
        (multiple of 128, max across cores, so the SPMD program is static);
      - gather (SWDGE queue 0): SBUF-source dma_gather of src rows ->
        G [128ci, n_k] bf16 (pads fetch the zero row);
      - matmul per 128-token block: Z[tok, co] = G_blk^T @ W[k] into PSUM
        (4 blocks per PSUM tile, one ACT copy to the SBUF stage);
      - dma_scatter_add f32 (SWDGE queue 1 - separate queue so the two
        SWDGE streams overlap) into one of 3 DRAM accumulators, assigned
        by k-group. Within one k the dst rows are unique so descriptors
        never race; same-buffer instructions are ordered by the tile
        framework + queue FIFO.
  * The self offset (k=13) is dense: per supertile, PE-transpose the
    table rows, Z_self^T = W[13]^T @ X^T, transpose back, and write as
    accumulator 0's initializer (BatchNorm cancels the conv bias b).
    Buffers 1, 2 are zero-filled.
  * k-group buffers let the tail start early: buffer g is complete once
    group g's scatters drain, so its readback + combine into bf16 SBUF
    tiles overlaps the remaining groups' gather/scatter stream. Only the
    last group's readback + stats (per-channel sum/sumsq via ones-matmul),
    the 1KB AllReduce, and the BN apply (DVE affine + ACT Prelu
    alpha=0.333) sit after the final scatter.

Falls back to a pure-numpy reference computation if the input graph is not
separable into <=25088-row shards (never the case for the intended input
distribution).
"""

import numpy as np
import ml_dtypes

C = 128
K = 27
EPS = 1e-4
LEAK = 0.333
N_CORES = 8
SELF_K = 13
ST = 512                 # supertile rows (self/tail phases)
SHARD = 25_088           # padded rows per core (49 supertiles)
N_ST = SHARD // ST
TABLE_ROWS = 25_216      # shard table rows incl. zero pad
ZERO_ROW = TABLE_ROWS - 1
DUMP_ROW = TABLE_ROWS - 1   # scatter pad target (beyond SHARD)
N_TOTAL = 200_000
KG_LIST = [k for k in range(K) if k != SELF_K]
GCHUNK = 512             # max idxs per single-packet SWDGE instruction


class Plan:
    """Static per-build parameters shared by all cores (SPMD program)."""

    def __init__(self, budgets):
        assert len(budgets) == len(KG_LIST)
        self.budgets = budgets            # per-k token budget, mult of 128
        self.n_cores = N_CORES
        self.n_total = N_TOTAL
        self.nbuf = 3            # k-groups (early readback granularity)
        self.sub_bufs = 2        # accumulators per group, chunk round-robin
        self.nqueues = 2
        self.gather_queues = [0]
        self.scatter_queues = [1]

    @property
    def total_tokens(self):
        return sum(self.budgets)

    def chunks(self, k_i):
        b = self.budgets[k_i]
        out = [GCHUNK] * (b // GCHUNK)
        if b % GCHUNK:
            out.append(b % GCHUNK)
        return out

    def group_of(self, ki):
        nkg = len(KG_LIST)
        return min(ki * self.nbuf // nkg, self.nbuf - 1)


def _partition_components(nb, n, n_cores, shard_cap):
    """Whole-component LPT partition. Returns (members_per_core, ok)."""
    import scipy.sparse as sp
    import scipy.sparse.csgraph as csg
    import heapq

    valid = nb >= 0
    ii, kk = np.nonzero(valid)
    jj = nb[ii, kk]
    m = kk != SELF_K
    g = sp.coo_matrix((np.ones(m.sum(), np.int8), (ii[m], jj[m])), shape=(n, n))
    _, labels = csg.connected_components(g, directed=False)
    sizes = np.bincount(labels)
    if sizes.max() > shard_cap:
        return None, False
    order = np.argsort(sizes)[::-1]
    heap = [(0, c) for c in range(n_cores)]
    heapq.heapify(heap)
    assign = np.empty(len(sizes), np.int32)
    for comp in order:
        load, c = heapq.heappop(heap)
        assign[comp] = c
        heapq.heappush(heap, (load + int(sizes[comp]), c))
    if max(l for l, _ in heap) > shard_cap:
        return None, False
    shard_of = assign[labels]
    members = [np.nonzero(shard_of == c)[0] for c in range(n_cores)]
    return members, True


def _wrap_idx(idx16):
    """k-major token list [n] int16 -> [128, n/16] wrapped + replicated."""
    n = idx16.shape[0]
    assert n % 16 == 0
    wrapped = idx16.reshape(n // 16, 16).T          # [16, cols]
    return np.ascontiguousarray(np.tile(wrapped, (8, 1)))


def _make_plan(nb, members):
    """Per-k budgets = max over cores of the valid-pair count, rounded up."""
    budgets = []
    counts = np.zeros((len(members), len(KG_LIST)), np.int64)
    for ci, mem in enumerate(members):
        nbm = nb[mem]
        for ki, k in enumerate(KG_LIST):
            counts[ci, ki] = int((nbm[:, k] >= 0).sum())
    for ki in range(len(KG_LIST)):
        b = int(counts[:, ki].max())
        b = max(128, ((b + 127) // 128) * 128)
        budgets.append(b)
    return Plan(budgets)


def _prepare_core_inputs(features, nb, members, plan):
    n = features.shape[0]
    loc = np.full(n, ZERO_ROW, np.int32)
    for mem in members:
        loc[mem] = np.arange(len(mem), dtype=np.int32)

    in_maps = []
    for mem in members:
        real = len(mem)
        assert real <= SHARD
        table = np.zeros((TABLE_ROWS, C), ml_dtypes.bfloat16)
        table[:real] = features[mem].astype(ml_dtypes.bfloat16)

        nb_loc = nb[mem]                                  # [real, K]
        g_list, s_list = [], []
        for ki, k in enumerate(KG_LIST):
            col = nb_loc[:, k]
            ii = np.nonzero(col >= 0)[0]                  # dst rows (unique)
            jj = loc[col[ii]]                             # src rows, local
            assert (jj < real).all(), "neighbor escaped shard"
            b = plan.budgets[ki]
            assert len(ii) <= b, (len(ii), b)
            gpad = np.full(b, ZERO_ROW, np.int32)
            spad = np.full(b, DUMP_ROW, np.int32)
            gpad[:len(jj)] = jj
            spad[:len(ii)] = ii
            g_list.append(gpad)
            s_list.append(spad)
        g_idx = np.concatenate(g_list).astype(np.int16)
        s_idx = np.concatenate(s_list).astype(np.int16)
        in_maps.append({
            "table": table,
            "gidx": _wrap_idx(g_idx),
            "sidx": _wrap_idx(s_idx),
        })
    return in_maps


def emit_kernel(tc, out_ap, ins, plan):
    import concourse.mybir as mybir
    from concourse.bass import ts
    from concourse.masks import make_identity

    nc = tc.nc
    F32 = mybir.dt.float32
    BF16 = mybir.dt.bfloat16
    I16 = mybir.dt.int16
    NKG = len(KG_LIST)
    NB = ST // 128
    LOOKAHEAD = 4
    NBUF = plan.nbuf

    table = ins["table"]
    gidx, sidx, w = ins["gidx"], ins["sidx"], ins["w"]
    gamma, beta = ins["gamma"], ins["beta"]
    idx_cols = plan.total_tokens // 16
    koff = np.cumsum([0] + plan.budgets).tolist()
    # last ki of each k-group (buffer g completes after this ki's scatters)
    group_last = {}
    for ki in range(NKG):
        group_last[plan.group_of(ki)] = ki

    with (
        tc.tile_pool(name="const", bufs=1) as constp,
        tc.tile_pool(name="dram", bufs=1, space="DRAM") as dramp,
    ):
        # ---- constants ----
        table_sb = constp.tile([128, (TABLE_ROWS // 128) * C], BF16)
        nc.sync.dma_start(
            table_sb[:].rearrange("p (r c) -> p r c", r=TABLE_ROWS // 128),
            table.rearrange("(r p) c -> p r c", p=128))
        w_sb = constp.tile([128, K * C], BF16)
        nc.sync.dma_start(w_sb[:].rearrange("ci (k co) -> ci k co", k=K),
                          w.rearrange("k ci co -> ci k co"))
        gidx_sb = constp.tile([128, idx_cols], I16)
        nc.sync.dma_start(gidx_sb[:], gidx)
        sidx_sb = constp.tile([128, idx_cols], I16)
        nc.sync.dma_start(sidx_sb[:], sidx)
        gamma_sb = constp.tile([1, C], F32)
        nc.sync.dma_start(gamma_sb[:], gamma[None, :])
        beta_sb = constp.tile([1, C], F32)
        nc.sync.dma_start(beta_sb[:], beta[None, :])
        identity = constp.tile([128, 128], F32)
        make_identity(nc, identity[:])
        identity_bf = constp.tile([128, 128], BF16)
        nc.vector.tensor_copy(identity_bf[:], identity[:])
        ones_sb = constp.tile([128, 1], BF16)
        nc.vector.memset(ones_sb[:], 1.0)
        zero128 = constp.tile([128, 1, C], F32)
        nc.vector.memset(zero128[:], 0.0)
        zero_st = constp.tile([128, NB, C], F32)
        nc.vector.memset(zero_st[:], 0.0)

        SUBS = getattr(plan, "sub_bufs", 2)
        pres = [[dramp.tile([TABLE_ROWS, C], F32, name=f"pre{i}_{j}")
                 for j in range(SUBS)] for i in range(NBUF)]

        # ---- self offset: pres[0][r] = W[13]^T x[r]; zero pres[1:] ----
        with (
            tc.tile_pool(name="selfw", bufs=3) as selfw,
            tc.tile_pool(name="selfps", bufs=2, space="PSUM") as selfps,
        ):
            for s in range(N_ST):
                xtp = selfps.tile([128, ST], BF16)
                for b in range(NB):
                    nc.tensor.transpose(xtp[:, ts(b, 128)],
                                        table_sb[:, ts(NB * s + b, C)],
                                        identity_bf[:])
                xcol = selfw.tile([128, ST], BF16)
                nc.vector.tensor_copy(xcol[:], xtp[:])
                ps = selfps.tile([128, ST], F32)
                nc.tensor.matmul(ps[:], w_sb[:, ts(SELF_K, C)], xcol[:],
                                 start=True, stop=True)
                zs = selfw.tile([128, ST], F32)
                nc.scalar.copy(zs[:], ps[:])
                pt = selfps.tile([128, ST], F32)
                for b in range(NB):
                    nc.tensor.transpose(pt[:, ts(b, 128)], zs[:, ts(b, 128)],
                                        identity[:])
                stage = selfw.tile([128, ST], F32)
                nc.vector.tensor_copy(stage[:], pt[:])
                nc.sync.dma_start(
                    pres[0][0][s * ST:(s + 1) * ST, :].rearrange(
                        "(b p) c -> p b c", p=128),
                    stage[:].rearrange("p (b c) -> p b c", b=NB))
                for grp in pres:
                    for pz in grp:
                        if pz is pres[0][0]:
                            continue
                        nc.sync.dma_start(
                            pz[s * ST:(s + 1) * ST, :].rearrange(
                                "(b p) c -> p b c", p=128),
                            zero_st[:])
            for grp in pres:
                for pz in grp:
                    nc.sync.dma_start(
                        pz[SHARD:TABLE_ROWS, :].rearrange("(b p) c -> p b c",
                                                          p=128),
                        zero128[:])

        # ---- main: per-offset gather -> matmul -> scatter-add;
        #      per-group early readback + combine into bf16 SBUF tiles ----
        xbpool = tc.alloc_tile_pool(name="xbpool", bufs=N_ST)
        xbtiles = [None] * N_ST
        statps = tc.alloc_tile_pool(name="statps", bufs=1, space="PSUM")
        sum_ps = statps.tile([1, ST], F32)
        sq_ps = statps.tile([1, ST], F32)

        with (
            tc.tile_pool(name="gath", bufs=LOOKAHEAD + 1) as gathp,
            tc.tile_pool(name="zst", bufs=3) as zstp,
            tc.tile_pool(name="psum", bufs=4, space="PSUM") as psump,
            tc.tile_pool(name="statw", bufs=4) as statw,
        ):
            gprobe = None
            if getattr(plan, "skip_gather", False):  # perf probe only
                gprobe = constp.tile([128, 1, max(plan.budgets)], BF16)
                nc.vector.memset(gprobe[:], 0)

            gqueues = plan.gather_queues
            gq_count = [0]

            def do_gather(ki):
                if gprobe is not None:  # perf probe only
                    return gprobe
                b = plan.budgets[ki]
                g = gathp.tile([128, 1, b], BF16)
                off = koff[ki]
                pos = 0
                for ch in plan.chunks(ki):
                    q = gqueues[gq_count[0] % len(gqueues)]
                    gq_count[0] += 1
                    nc.gpsimd.dma_gather(
                        g[:, :, pos:pos + ch], table_sb[:],
                        gidx_sb[:, (off + pos) // 16:(off + pos + ch) // 16],
                        ch, ch, C,
                        transpose=True, single_packet=True,
                        sbuf_tokens_per_rank=128,
                        sbuf_free_dim_per_rank=C * 2,
                        queue_num=q)
                    pos += ch
                return g

            def readback_group(grp):
                """Group `grp`'s sub-buffers are complete: read them back and
                fold into the bf16 combine tiles (last group also feeds the
                stats)."""
                last = (grp == NBUF - 1)
                for s in range(N_ST):
                    xts = []
                    for pz in pres[grp]:
                        xt = statw.tile([128, NB, C], F32)
                        nc.sync.dma_start(
                            xt[:], pz[s * ST:(s + 1) * ST, :].rearrange(
                                "(b p) c -> p b c", p=128))
                        xts.append(xt)
                    if grp == 0:
                        xb = xbpool.tile([128, NB * C], BF16)
                        if len(xts) == 1:
                            nc.vector.tensor_copy(
                                xb[:], xts[0][:].rearrange("p b c -> p (b c)"))
                        else:
                            nc.vector.tensor_tensor(
                                out=xb[:],
                                in0=xts[0][:].rearrange("p b c -> p (b c)"),
                                in1=xts[1][:].rearrange("p b c -> p (b c)"),
                                op=mybir.AluOpType.add)
                        xbtiles[s] = xb
                    else:
                        xb = xbtiles[s]
                        for xt in xts:
                            nc.vector.tensor_tensor(
                                out=xb[:], in0=xb[:],
                                in1=xt[:].rearrange("p b c -> p (b c)"),
                                op=mybir.AluOpType.add)
                    if last:
                        sq = statw.tile([128, NB * C], BF16)
                        nc.scalar.activation(
                            sq[:], xb[:], mybir.ActivationFunctionType.Square)
                        nc.tensor.matmul(sum_ps[:], ones_sb[:], xb[:],
                                         start=(s == 0), stop=(s == N_ST - 1))
                        nc.tensor.matmul(sq_ps[:], ones_sb[:], sq[:],
                                         start=(s == 0), stop=(s == N_ST - 1))

            gtiles = {}
            sc_count = [0]
            for ki in range(min(LOOKAHEAD, NKG)):
                gtiles[ki] = do_gather(ki)

            for ki, k in enumerate(KG_LIST):
                b = plan.budgets[ki]
                nblk = b // 128
                g = gtiles.pop(ki)
                zst = zstp.tile([128, nblk, C], F32)
                for g0 in range(0, nblk, 4):
                    g1 = min(g0 + 4, nblk)
                    zp = psump.tile([128, 4, C], F32)
                    for blk in range(g0, g1):
                        nc.tensor.matmul(zp[:, blk - g0, :],
                                         g[:, 0, ts(blk, 128)],
                                         w_sb[:, ts(k, C)],
                                         start=True, stop=True)
                    nc.scalar.copy(zst[:, g0:g1, :], zp[:, 0:g1 - g0, :])
                off = koff[ki]
                pos = 0
                grp_bufs = pres[plan.group_of(ki)]
                squeues = plan.scatter_queues
                for ch in plan.chunks(ki):
                    if getattr(plan, "skip_scatter", False):  # perf probe
                        break
                    q = squeues[sc_count[0] % len(squeues)]
                    dst = grp_bufs[sc_count[0] % len(grp_bufs)]
                    sc_count[0] += 1
                    nc.gpsimd.dma_scatter_add(
                        dst[:, :], zst[:, pos // 128:(pos + ch) // 128, :],
                        sidx_sb[:, (off + pos) // 16:(off + pos + ch) // 16],
                        ch, ch, C, single_packet=True, queue_num=q)
                    pos += ch
                if ki + LOOKAHEAD < NKG:
                    gtiles[ki + LOOKAHEAD] = do_gather(ki + LOOKAHEAD)
                for grp, lki in group_last.items():
                    if lki == ki:
                        readback_group(grp)

        # ---- stats finalize -> allreduce -> BN coefficients ----
        stats_sb = constp.tile([1, 2 * C], F32)
        acc = constp.tile([1, 2 * C], F32)
        nc.vector.tensor_copy(acc[:, 0:C], sum_ps[:, 0:C])
        nc.vector.tensor_copy(acc[:, C:2 * C], sq_ps[:, 0:C])
        for b in range(1, NB):
            nc.vector.tensor_tensor(out=acc[:, 0:C], in0=acc[:, 0:C],
                                    in1=sum_ps[:, ts(b, C)],
                                    op=mybir.AluOpType.add)
            nc.vector.tensor_tensor(out=acc[:, C:2 * C], in0=acc[:, C:2 * C],
                                    in1=sq_ps[:, ts(b, C)],
                                    op=mybir.AluOpType.add)
        nc.vector.tensor_copy(stats_sb[:], acc[:])
        statps.release()

        if plan.n_cores > 1 and not getattr(plan, "skip_collective", False):
            stats_in = dramp.tile([1, 2 * C], F32)
            stats_out = dramp.tile([1, 2 * C], F32)
            nc.sync.dma_start(stats_in[:], stats_sb[:])
            nc.gpsimd.collective_compute(
                "AllReduce", mybir.AluOpType.add,
                replica_groups=[list(range(plan.n_cores))],
                ins=[stats_in.opt()], outs=[stats_out.opt()],
            )
            stats2 = constp.tile([1, 2 * C], F32)
            nc.sync.dma_start(stats2[:], stats_out[:])
        else:
            stats2 = stats_sb

        mean_t = constp.tile([1, C], F32)
        ex2_t = constp.tile([1, C], F32)
        var_t = constp.tile([1, C], F32)
        std_t = constp.tile([1, C], F32)
        rstd_t = constp.tile([1, C], F32)
        s_vec = constp.tile([1, C], F32)
        t_vec = constp.tile([1, C], F32)
        tmp = constp.tile([1, C], F32)
        inv_n = 1.0 / plan.n_total
        nc.vector.tensor_scalar_mul(mean_t[:], stats2[:, 0:C], inv_n)
        nc.vector.tensor_scalar_mul(ex2_t[:], stats2[:, C:2 * C], inv_n)
        nc.vector.tensor_tensor(out=tmp[:], in0=mean_t[:], in1=mean_t[:],
                                op=mybir.AluOpType.mult)
        nc.vector.tensor_tensor(out=var_t[:], in0=ex2_t[:], in1=tmp[:],
                                op=mybir.AluOpType.subtract)
        nc.vector.tensor_scalar_add(var_t[:], var_t[:], EPS)
        nc.scalar.activation(std_t[:], var_t[:],
                             mybir.ActivationFunctionType.Sqrt)
        nc.vector.reciprocal(rstd_t[:], std_t[:])
        nc.vector.tensor_tensor(out=s_vec[:], in0=rstd_t[:], in1=gamma_sb[:],
                                op=mybir.AluOpType.mult)
        nc.vector.tensor_tensor(out=tmp[:], in0=mean_t[:], in1=s_vec[:],
                                op=mybir.AluOpType.mult)
        nc.vector.tensor_tensor(out=t_vec[:], in0=beta_sb[:], in1=tmp[:],
                                op=mybir.AluOpType.subtract)

        s_bc = constp.tile([128, NB, C], F32)
        t_bc = constp.tile([128, NB, C], F32)
        nc.gpsimd.partition_broadcast(s_bc[:, 0, :], s_vec[:])
        nc.gpsimd.partition_broadcast(t_bc[:, 0, :], t_vec[:])
        for b in range(1, NB):
            nc.vector.tensor_copy(s_bc[:, b, :], s_bc[:, 0, :])
            nc.vector.tensor_copy(t_bc[:, b, :], t_bc[:, 0, :])

        # ---- BN apply + LeakyReLU + writeback ----
        with tc.tile_pool(name="applyw", bufs=3) as applyp:
            for s in range(N_ST):
                xb = xbtiles[s]
                y = applyp.tile([128, NB, C], F32)
                nc.vector.tensor_tensor(
                    out=y[:].rearrange("p b c -> p (b c)"), in0=xb[:],
                    in1=s_bc[:].rearrange("p b c -> p (b c)"),
                    op=mybir.AluOpType.mult)
                nc.vector.tensor_tensor(out=y[:], in0=y[:], in1=t_bc[:],
                                        op=mybir.AluOpType.add)
                yo = applyp.tile([128, NB, C], F32)
                nc.scalar.activation(yo[:].rearrange("p b c -> p (b c)"),
                                     y[:].rearrange("p b c -> p (b c)"),
                                     mybir.ActivationFunctionType.Prelu,
                                     alpha=LEAK)
                nc.sync.dma_start(
                    out_ap[s * ST:(s + 1) * ST, :].rearrange(
                        "(b p) c -> p b c", p=128),
                    yo[:])
        xbpool.release()


def _build_bass(plan, reps=1):
    import concourse.bacc as bacc
    import concourse.mybir as mybir
    import concourse.tile as tile

    nc = bacc.Bacc("TRN2", target_bir_lowering=False, debug=False,
                   num_devices=plan.n_cores,
                   num_swdge_queues=getattr(plan, "nqueues", 2))
    F32 = mybir.dt.float32
    BF16 = mybir.dt.bfloat16
    I16 = mybir.dt.int16
    idx_cols = plan.total_tokens // 16
    ins = {
        "table": nc.dram_tensor("table", [TABLE_ROWS, C], BF16,
                                kind="ExternalInput")[:, :],
        "gidx": nc.dram_tensor("gidx", [128, idx_cols], I16,
                               kind="ExternalInput")[:, :],
        "sidx": nc.dram_tensor("sidx", [128, idx_cols], I16,
                               kind="ExternalInput")[:, :],
        "w": nc.dram_tensor("w", [K, C, C], BF16, kind="ExternalInput")[:, :, :],
        "gamma": nc.dram_tensor("gamma", [C], F32, kind="ExternalInput")[:],
        "beta": nc.dram_tensor("beta", [C], F32, kind="ExternalInput")[:],
    }
    out = nc.dram_tensor("out", [SHARD, C], F32, kind="ExternalOutput")
    with tile.TileContext(nc) as tc:
        for _ in range(reps):
            emit_kernel(tc, out[:, :], ins, plan)
    nc.compile()
    return nc


def _reference_fallback(features, w, b, gamma, beta, nb):
    feats = np.asarray(features, np.float32)
    wf = np.asarray(w, np.float32)
    out = np.broadcast_to(np.asarray(b, np.float32), feats.shape).copy()
    valid = nb >= 0
    idx = np.where(valid, nb, 0)
    for k in range(K):
        xk = feats[idx[:, k]] * valid[:, k:k + 1]
        out += xk @ wf[k]
    mean = out.mean(0)
    var = out.var(0)
    out = (out - mean) / np.sqrt(var + EPS) * np.asarray(gamma, np.float32) \
        + np.asarray(beta, np.float32)
    return np.where(out > 0, out, LEAK * out).astype(np.float32)


def kernel(features, W, b, gamma, beta, neighbor_idx):
    from concourse.bass_utils import run_bass_kernel_spmd

    features = np.asarray(features, np.float32)
    Wf = np.asarray(W, np.float32)
    gamma_f = np.asarray(gamma, np.float32)
    beta_f = np.asarray(beta, np.float32)
    nb = np.asarray(neighbor_idx, np.int32)
    assert features.shape == (N_TOTAL, C)

    members, ok = _partition_components(nb, N_TOTAL, N_CORES, SHARD)
    if not ok:
        return _reference_fallback(features, Wf, b, gamma_f, beta_f, nb)

    plan = _make_plan(nb, members)
    core_maps = _prepare_core_inputs(features, nb, members, plan)
    w_bf = Wf.astype(ml_dtypes.bfloat16)
    for m in core_maps:
        m["w"] = w_bf
        m["gamma"] = gamma_f
        m["beta"] = beta_f

    nc = _build_bass(plan)
    res = run_bass_kernel_spmd(nc, core_maps, core_ids=list(range(N_CORES)))

    out_full = np.empty((N_TOTAL, C), np.float32)
    for c, mem in enumerate(members):
        out_full[mem] = res.results[c]["out"][:len(mem)]
    return out_full


# revision 52
# speedup vs baseline: 1.5813x; 1.3913x over previous
"""Submanifold 3x3x3 sparse conv (gnn_message_passing) + BatchNorm + LeakyReLU
on 8 Trainium2 NeuronCores.

Strategy (N=200000, C=128, K=27, GRID=128^3 @ ~9.5% occupancy):
  * Whole-component LPT partition across the 8 cores: every neighbor
    reference stays inside its core's shard; shard-local indices fit in
    int16 for the SWDGE gather/scatter ucode.
  * At 9.5% occupancy only ~2.44 of the 26 non-self neighbor slots are
    valid, and the SWDGE path runs at a fixed per-token rate regardless of
    chunking, so this kernel only pays for valid pairs (vs 26*N for the
    dense formulation):
      - host builds, per offset k != 13, the compact list of valid
        (dst_row, src_row) pairs, padded to a shared per-k budget
        (multiple of 128, max across cores, so the SPMD program is static);
      - gather (SWDGE queue 0): SBUF-source dma_gather of src rows ->
        G [128ci, n_k] bf16 (pads fetch the zero row);
      - matmul per 128-token block: Z[tok, co] = G_blk^T @ W[k] into PSUM
        (4 blocks per PSUM tile, one ACT copy to the SBUF stage);
      - dma_scatter_add f32 (SWDGE queue 1 - separate queue so the two
        SWDGE streams overlap) into one of 3 DRAM accumulators, assigned
        by k-group. Within one k the dst rows are unique so descriptors
        never race; same-buffer instructions are ordered by the tile
        framework + queue FIFO.
  * The self offset (k=13) is dense: per supertile, PE-transpose the
    table rows, Z_self^T = W[13]^T @ X^T, transpose back, and write as
    accumulator 0's initializer (BatchNorm cancels the conv bias b).
    Buffers 1, 2 are zero-filled.
  * k-group buffers let the tail start early: buffer g is complete once
    group g's scatters drain, so its readback + combine into bf16 SBUF
    tiles overlaps the remaining groups' gather/scatter stream. Only the
    last group's readback + stats (per-channel sum/sumsq via ones-matmul),
    the 1KB AllReduce, and the BN apply (DVE affine + ACT Prelu
    alpha=0.333) sit after the final scatter.

Falls back to a pure-numpy reference computation if the input graph is not
separable into <=25088-row shards (never the case for the intended input
distribution).
"""

import numpy as np
import ml_dtypes

C = 128
K = 27
EPS = 1e-4
LEAK = 0.333
N_CORES = 8
SELF_K = 13
ST = 512                 # supertile rows (self/tail phases)
SHARD = 25_088           # padded rows per core (49 supertiles)
N_ST = SHARD // ST
TABLE_ROWS = 25_216      # shard table rows incl. zero pad
ZERO_ROW = TABLE_ROWS - 1
DUMP_ROW = TABLE_ROWS - 1   # scatter pad target (beyond SHARD)
N_TOTAL = 200_000
KG_LIST = [k for k in range(K) if k != SELF_K]
GCHUNK = 512             # max idxs per single-packet SWDGE instruction


class Plan:
    """Static per-build parameters shared by all cores (SPMD program)."""

    def __init__(self, budgets):
        assert len(budgets) == len(KG_LIST)
        self.budgets = budgets            # per-k token budget, mult of 128
        self.n_cores = N_CORES
        self.n_total = N_TOTAL
        self.nbuf = 3            # k-groups (early readback granularity)
        self.sub_bufs = 2        # accumulators per group, chunk round-robin
        self.nqueues = 2
        self.gather_queues = [0]
        self.scatter_queues = [1]

    @property
    def total_tokens(self):
        return sum(self.budgets)

    def chunks(self, k_i):
        b = self.budgets[k_i]
        out = [GCHUNK] * (b // GCHUNK)
        if b % GCHUNK:
            out.append(b % GCHUNK)
        return out

    def group_of(self, ki):
        nkg = len(KG_LIST)
        return min(ki * self.nbuf // nkg, self.nbuf - 1)


def _partition_components(nb, n, n_cores, shard_cap):
    """Whole-component LPT partition. Returns (members_per_core, ok)."""
    import scipy.sparse as sp
    import scipy.sparse.csgraph as csg
    import heapq

    valid = nb >= 0
    ii, kk = np.nonzero(valid)
    jj = nb[ii, kk]
    m = kk != SELF_K
    g = sp.coo_matrix((np.ones(m.sum(), np.int8), (ii[m], jj[m])), shape=(n, n))
    _, labels = csg.connected_components(g, directed=False)
    sizes = np.bincount(labels)
    if sizes.max() > shard_cap:
        return None, False
    order = np.argsort(sizes)[::-1]
    heap = [(0, c) for c in range(n_cores)]
    heapq.heapify(heap)
    assign = np.empty(len(sizes), np.int32)
    for comp in order:
        load, c = heapq.heappop(heap)
        assign[comp] = c
        heapq.heappush(heap, (load + int(sizes[comp]), c))
    if max(l for l, _ in heap) > shard_cap:
        return None, False
    shard_of = assign[labels]
    members = [np.nonzero(shard_of == c)[0] for c in range(n_cores)]
    return members, True


def _wrap_idx(idx16):
    """k-major token list [n] int16 -> [128, n/16] wrapped + replicated."""
    n = idx16.shape[0]
    assert n % 16 == 0
    wrapped = idx16.reshape(n // 16, 16).T          # [16, cols]
    return np.ascontiguousarray(np.tile(wrapped, (8, 1)))


def _make_plan(nb, members):
    """Per-k budgets = max over cores of the valid-pair count, rounded up."""
    budgets = []
    counts = np.zeros((len(members), len(KG_LIST)), np.int64)
    for ci, mem in enumerate(members):
        nbm = nb[mem]
        for ki, k in enumerate(KG_LIST):
            counts[ci, ki] = int((nbm[:, k] >= 0).sum())
    for ki in range(len(KG_LIST)):
        b = int(counts[:, ki].max())
        b = max(128, ((b + 127) // 128) * 128)
        budgets.append(b)
    return Plan(budgets)


def _prepare_core_inputs(features, nb, members, plan):
    n = features.shape[0]
    loc = np.full(n, ZERO_ROW, np.int32)
    for mem in members:
        loc[mem] = np.arange(len(mem), dtype=np.int32)

    in_maps = []
    for mem in members:
        real = len(mem)
        assert real <= SHARD
        table = np.zeros((TABLE_ROWS, C), ml_dtypes.bfloat16)
        table[:real] = features[mem].astype(ml_dtypes.bfloat16)

        nb_loc = nb[mem]                                  # [real, K]
        g_list, s_list = [], []
        for ki, k in enumerate(KG_LIST):
            col = nb_loc[:, k]
            ii = np.nonzero(col >= 0)[0]                  # dst rows (unique)
            jj = loc[col[ii]]                             # src rows, local
            assert (jj < real).all(), "neighbor escaped shard"
            b = plan.budgets[ki]
            assert len(ii) <= b, (len(ii), b)
            gpad = np.full(b, ZERO_ROW, np.int32)
            spad = np.full(b, DUMP_ROW, np.int32)
            gpad[:len(jj)] = jj
            spad[:len(ii)] = ii
            g_list.append(gpad)
            s_list.append(spad)
        g_idx = np.concatenate(g_list).astype(np.int16)
        s_idx = np.concatenate(s_list).astype(np.int16)
        in_maps.append({
            "table": table,
            "gidx": _wrap_idx(g_idx),
            "sidx": _wrap_idx(s_idx),
        })
    return in_maps


def emit_kernel(tc, out_ap, ins, plan):
    import concourse.mybir as mybir
    from concourse.bass import ts
    from concourse.masks import make_identity

    nc = tc.nc
    F32 = mybir.dt.float32
    BF16 = mybir.dt.bfloat16
    I16 = mybir.dt.int16
    NKG = len(KG_LIST)
    NB = ST // 128
    LOOKAHEAD = 4
    NBUF = plan.nbuf

    table = ins["table"]
    gidx, sidx, w = ins["gidx"], ins["sidx"], ins["w"]
    gamma, beta = ins["gamma"], ins["beta"]
    idx_cols = plan.total_tokens // 16
    koff = np.cumsum([0] + plan.budgets).tolist()
    # last ki of each k-group (buffer g completes after this ki's scatters)
    group_last = {}
    for ki in range(NKG):
        group_last[plan.group_of(ki)] = ki

    with (
        tc.tile_pool(name="const", bufs=1) as constp,
        tc.tile_pool(name="dram", bufs=1, space="DRAM") as dramp,
    ):
        # ---- constants ----
        table_sb = constp.tile([128, (TABLE_ROWS // 128) * C], BF16)
        nc.sync.dma_start(
            table_sb[:].rearrange("p (r c) -> p r c", r=TABLE_ROWS // 128),
            table.rearrange("(r p) c -> p r c", p=128))
        w_sb = constp.tile([128, K * C], BF16)
        nc.sync.dma_start(w_sb[:].rearrange("ci (k co) -> ci k co", k=K),
                          w.rearrange("k ci co -> ci k co"))
        gidx_sb = constp.tile([128, idx_cols], I16)
        nc.sync.dma_start(gidx_sb[:], gidx)
        sidx_sb = constp.tile([128, idx_cols], I16)
        nc.sync.dma_start(sidx_sb[:], sidx)
        gamma_sb = constp.tile([1, C], F32)
        nc.sync.dma_start(gamma_sb[:], gamma[None, :])
        beta_sb = constp.tile([1, C], F32)
        nc.sync.dma_start(beta_sb[:], beta[None, :])
        identity = constp.tile([128, 128], F32)
        make_identity(nc, identity[:])
        identity_bf = constp.tile([128, 128], BF16)
        nc.vector.tensor_copy(identity_bf[:], identity[:])
        ones_sb = constp.tile([128, 1], BF16)
        nc.vector.memset(ones_sb[:], 1.0)
        zero128 = constp.tile([128, 1, C], F32)
        nc.vector.memset(zero128[:], 0.0)
        zero_st = constp.tile([128, NB, C], F32)
        nc.vector.memset(zero_st[:], 0.0)

        SUBS = getattr(plan, "sub_bufs", 2)
        pres = [[dramp.tile([TABLE_ROWS, C], F32, name=f"pre{i}_{j}")
                 for j in range(SUBS)] for i in range(NBUF)]

        # ---- self offset: pres[0][r] = W[13]^T x[r]; zero pres[1:] ----
        with (
            tc.tile_pool(name="selfw", bufs=3) as selfw,
            tc.tile_pool(name="selfps", bufs=2, space="PSUM") as selfps,
        ):
            for s in range(N_ST):
                xtp = selfps.tile([128, ST], BF16)
                for b in range(NB):
                    nc.tensor.transpose(xtp[:, ts(b, 128)],
                                        table_sb[:, ts(NB * s + b, C)],
                                        identity_bf[:])
                xcol = selfw.tile([128, ST], BF16)
                nc.vector.tensor_copy(xcol[:], xtp[:])
                ps = selfps.tile([128, ST], F32)
                nc.tensor.matmul(ps[:], w_sb[:, ts(SELF_K, C)], xcol[:],
                                 start=True, stop=True)
                zs = selfw.tile([128, ST], F32)
                nc.scalar.copy(zs[:], ps[:])
                pt = selfps.tile([128, ST], F32)
                for b in range(NB):
                    nc.tensor.transpose(pt[:, ts(b, 128)], zs[:, ts(b, 128)],
                                        identity[:])
                stage = selfw.tile([128, ST], F32)
                nc.vector.tensor_copy(stage[:], pt[:])
                nc.sync.dma_start(
                    pres[0][0][s * ST:(s + 1) * ST, :].rearrange(
                        "(b p) c -> p b c", p=128),
                    stage[:].rearrange("p (b c) -> p b c", b=NB))
                for grp in pres:
                    for pz in grp:
                        if pz is pres[0][0]:
                            continue
                        nc.sync.dma_start(
                            pz[s * ST:(s + 1) * ST, :].rearrange(
                                "(b p) c -> p b c", p=128),
                            zero_st[:])
            for grp in pres:
                for pz in grp:
                    nc.sync.dma_start(
                        pz[SHARD:TABLE_ROWS, :].rearrange("(b p) c -> p b c",
                                                          p=128),
                        zero128[:])

        # ---- main: per-offset gather -> matmul -> scatter-add;
        #      per-group early readback + combine into bf16 SBUF tiles ----
        xbpool = tc.alloc_tile_pool(name="xbpool", bufs=N_ST)
        xbtiles = [None] * N_ST
        statps = tc.alloc_tile_pool(name="statps", bufs=1, space="PSUM")
        sum_ps = statps.tile([1, ST], F32)
        sq_ps = statps.tile([1, ST], F32)

        with (
            tc.tile_pool(name="gath", bufs=LOOKAHEAD + 1) as gathp,
            tc.tile_pool(name="zst", bufs=3) as zstp,
            tc.tile_pool(name="psum", bufs=4, space="PSUM") as psump,
            tc.tile_pool(name="statw", bufs=4) as statw,
        ):
            gprobe = None
            if getattr(plan, "skip_gather", False):  # perf probe only
                gprobe = constp.tile([128, 1, max(plan.budgets)], BF16)
                nc.vector.memset(gprobe[:], 0)

            gqueues = plan.gather_queues
            gq_count = [0]

            def do_gather(ki):
                if gprobe is not None:  # perf probe only
                    return gprobe
                b = plan.budgets[ki]
                g = gathp.tile([128, 1, b], BF16)
                off = koff[ki]
                pos = 0
                if getattr(plan, "gather_whole_k", False):
                    chunks = [b]
                else:
                    chunks = plan.chunks(ki)
                for ch in chunks:
                    q = gqueues[gq_count[0] % len(gqueues)]
                    gq_count[0] += 1
                    nc.gpsimd.dma_gather(
                        g[:, :, pos:pos + ch], table_sb[:],
                        gidx_sb[:, (off + pos) // 16:(off + pos + ch) // 16],
                        ch, ch, C,
                        transpose=True, single_packet=(ch <= GCHUNK),
                        sbuf_tokens_per_rank=128,
                        sbuf_free_dim_per_rank=C * 2,
                        queue_num=q)
                    pos += ch
                return g

            def readback_group(grp):
                """Group `grp`'s sub-buffers are complete: read them back and
                fold into the bf16 combine tiles (last group also feeds the
                stats)."""
                last = (grp == NBUF - 1)
                for s in range(N_ST):
                    xts = []
                    for pz in pres[grp]:
                        xt = statw.tile([128, NB, C], F32)
                        nc.sync.dma_start(
                            xt[:], pz[s * ST:(s + 1) * ST, :].rearrange(
                                "(b p) c -> p b c", p=128))
                        xts.append(xt)
                    if grp == 0:
                        xb = xbpool.tile([128, NB * C], BF16)
                        if len(xts) == 1:
                            nc.vector.tensor_copy(
                                xb[:], xts[0][:].rearrange("p b c -> p (b c)"))
                        else:
                            nc.vector.tensor_tensor(
                                out=xb[:],
                                in0=xts[0][:].rearrange("p b c -> p (b c)"),
                                in1=xts[1][:].rearrange("p b c -> p (b c)"),
                                op=mybir.AluOpType.add)
                        xbtiles[s] = xb
                    else:
                        xb = xbtiles[s]
                        for xt in xts:
                            nc.vector.tensor_tensor(
                                out=xb[:], in0=xb[:],
                                in1=xt[:].rearrange("p b c -> p (b c)"),
                                op=mybir.AluOpType.add)
                    if last:
                        sq = statw.tile([128, NB * C], BF16)
                        nc.scalar.activation(
                            sq[:], xb[:], mybir.ActivationFunctionType.Square)
                        nc.tensor.matmul(sum_ps[:], ones_sb[:], xb[:],
                                         start=(s == 0), stop=(s == N_ST - 1))
                        nc.tensor.matmul(sq_ps[:], ones_sb[:], sq[:],
                                         start=(s == 0), stop=(s == N_ST - 1))

            gtiles = {}
            sc_count = [0]
            for ki in range(min(LOOKAHEAD, NKG)):
                gtiles[ki] = do_gather(ki)

            for ki, k in enumerate(KG_LIST):
                b = plan.budgets[ki]
                nblk = b // 128
                g = gtiles.pop(ki)
                zst = zstp.tile([128, nblk, C], F32)
                for g0 in range(0, nblk, 4):
                    g1 = min(g0 + 4, nblk)
                    zp = psump.tile([128, 4, C], F32)
                    for blk in range(g0, g1):
                        nc.tensor.matmul(zp[:, blk - g0, :],
                                         g[:, 0, ts(blk, 128)],
                                         w_sb[:, ts(k, C)],
                                         start=True, stop=True)
                    nc.scalar.copy(zst[:, g0:g1, :], zp[:, 0:g1 - g0, :])
                off = koff[ki]
                pos = 0
                grp_bufs = pres[plan.group_of(ki)]
                squeues = plan.scatter_queues
                if getattr(plan, "scatter_whole_k", False):
                    sch = [b]
                else:
                    sch = plan.chunks(ki)
                for ch in sch:
                    if getattr(plan, "skip_scatter", False):  # perf probe
                        break
                    q = squeues[sc_count[0] % len(squeues)]
                    dst = grp_bufs[sc_count[0] % len(grp_bufs)]
                    sc_count[0] += 1
                    nc.gpsimd.dma_scatter_add(
                        dst[:, :], zst[:, pos // 128:(pos + ch) // 128, :],
                        sidx_sb[:, (off + pos) // 16:(off + pos + ch) // 16],
                        ch, ch, C, single_packet=(ch <= GCHUNK), queue_num=q)
                    pos += ch
                if ki + LOOKAHEAD < NKG:
                    gtiles[ki + LOOKAHEAD] = do_gather(ki + LOOKAHEAD)
                for grp, lki in group_last.items():
                    if lki == ki:
                        readback_group(grp)

        # ---- stats finalize -> allreduce -> BN coefficients ----
        stats_sb = constp.tile([1, 2 * C], F32)
        acc = constp.tile([1, 2 * C], F32)
        nc.vector.tensor_copy(acc[:, 0:C], sum_ps[:, 0:C])
        nc.vector.tensor_copy(acc[:, C:2 * C], sq_ps[:, 0:C])
        for b in range(1, NB):
            nc.vector.tensor_tensor(out=acc[:, 0:C], in0=acc[:, 0:C],
                                    in1=sum_ps[:, ts(b, C)],
                                    op=mybir.AluOpType.add)
            nc.vector.tensor_tensor(out=acc[:, C:2 * C], in0=acc[:, C:2 * C],
                                    in1=sq_ps[:, ts(b, C)],
                                    op=mybir.AluOpType.add)
        nc.vector.tensor_copy(stats_sb[:], acc[:])
        statps.release()

        if plan.n_cores > 1 and not getattr(plan, "skip_collective", False):
            stats_in = dramp.tile([1, 2 * C], F32)
            stats_out = dramp.tile([1, 2 * C], F32)
            nc.sync.dma_start(stats_in[:], stats_sb[:])
            nc.gpsimd.collective_compute(
                "AllReduce", mybir.AluOpType.add,
                replica_groups=[list(range(plan.n_cores))],
                ins=[stats_in.opt()], outs=[stats_out.opt()],
            )
            stats2 = constp.tile([1, 2 * C], F32)
            nc.sync.dma_start(stats2[:], stats_out[:])
        else:
            stats2 = stats_sb

        mean_t = constp.tile([1, C], F32)
        ex2_t = constp.tile([1, C], F32)
        var_t = constp.tile([1, C], F32)
        std_t = constp.tile([1, C], F32)
        rstd_t = constp.tile([1, C], F32)
        s_vec = constp.tile([1, C], F32)
        t_vec = constp.tile([1, C], F32)
        tmp = constp.tile([1, C], F32)
        inv_n = 1.0 / plan.n_total
        nc.vector.tensor_scalar_mul(mean_t[:], stats2[:, 0:C], inv_n)
        nc.vector.tensor_scalar_mul(ex2_t[:], stats2[:, C:2 * C], inv_n)
        nc.vector.tensor_tensor(out=tmp[:], in0=mean_t[:], in1=mean_t[:],
                                op=mybir.AluOpType.mult)
        nc.vector.tensor_tensor(out=var_t[:], in0=ex2_t[:], in1=tmp[:],
                                op=mybir.AluOpType.subtract)
        nc.vector.tensor_scalar_add(var_t[:], var_t[:], EPS)
        nc.scalar.activation(std_t[:], var_t[:],
                             mybir.ActivationFunctionType.Sqrt)
        nc.vector.reciprocal(rstd_t[:], std_t[:])
        nc.vector.tensor_tensor(out=s_vec[:], in0=rstd_t[:], in1=gamma_sb[:],
                                op=mybir.AluOpType.mult)
        nc.vector.tensor_tensor(out=tmp[:], in0=mean_t[:], in1=s_vec[:],
                                op=mybir.AluOpType.mult)
        nc.vector.tensor_tensor(out=t_vec[:], in0=beta_sb[:], in1=tmp[:],
                                op=mybir.AluOpType.subtract)

        s_bc = constp.tile([128, NB, C], F32)
        t_bc = constp.tile([128, NB, C], F32)
        nc.gpsimd.partition_broadcast(s_bc[:, 0, :], s_vec[:])
        nc.gpsimd.partition_broadcast(t_bc[:, 0, :], t_vec[:])
        for b in range(1, NB):
            nc.vector.tensor_copy(s_bc[:, b, :], s_bc[:, 0, :])
            nc.vector.tensor_copy(t_bc[:, b, :], t_bc[:, 0, :])

        # ---- BN apply + LeakyReLU + writeback ----
        with tc.tile_pool(name="applyw", bufs=3) as applyp:
            for s in range(N_ST):
                xb = xbtiles[s]
                y = applyp.tile([128, NB, C], F32)
                nc.vector.tensor_tensor(
                    out=y[:].rearrange("p b c -> p (b c)"), in0=xb[:],
                    in1=s_bc[:].rearrange("p b c -> p (b c)"),
                    op=mybir.AluOpType.mult)
                nc.vector.tensor_tensor(out=y[:], in0=y[:], in1=t_bc[:],
                                        op=mybir.AluOpType.add)
                yo = applyp.tile([128, NB, C], F32)
                nc.scalar.activation(yo[:].rearrange("p b c -> p (b c)"),
                                     y[:].rearrange("p b c -> p (b c)"),
                                     mybir.ActivationFunctionType.Prelu,
                                     alpha=LEAK)
                nc.sync.dma_start(
                    out_ap[s * ST:(s + 1) * ST, :].rearrange(
                        "(b p) c -> p b c", p=128),
                    yo[:])
        xbpool.release()


def _build_bass(plan, reps=1):
    import concourse.bacc as bacc
    import concourse.mybir as mybir
    import concourse.tile as tile

    nc = bacc.Bacc("TRN2", target_bir_lowering=False, debug=False,
                   num_devices=plan.n_cores,
                   num_swdge_queues=getattr(plan, "nqueues", 2))
    F32 = mybir.dt.float32
    BF16 = mybir.dt.bfloat16
    I16 = mybir.dt.int16
    idx_cols = plan.total_tokens // 16
    ins = {
        "table": nc.dram_tensor("table", [TABLE_ROWS, C], BF16,
                                kind="ExternalInput")[:, :],
        "gidx": nc.dram_tensor("gidx", [128, idx_cols], I16,
                               kind="ExternalInput")[:, :],
        "sidx": nc.dram_tensor("sidx", [128, idx_cols], I16,
                               kind="ExternalInput")[:, :],
        "w": nc.dram_tensor("w", [K, C, C], BF16, kind="ExternalInput")[:, :, :],
        "gamma": nc.dram_tensor("gamma", [C], F32, kind="ExternalInput")[:],
        "beta": nc.dram_tensor("beta", [C], F32, kind="ExternalInput")[:],
    }
    out = nc.dram_tensor("out", [SHARD, C], F32, kind="ExternalOutput")
    with tile.TileContext(nc) as tc:
        for _ in range(reps):
            emit_kernel(tc, out[:, :], ins, plan)
    nc.compile()
    return nc


def _reference_fallback(features, w, b, gamma, beta, nb):
    feats = np.asarray(features, np.float32)
    wf = np.asarray(w, np.float32)
    out = np.broadcast_to(np.asarray(b, np.float32), feats.shape).copy()
    valid = nb >= 0
    idx = np.where(valid, nb, 0)
    for k in range(K):
        xk = feats[idx[:, k]] * valid[:, k:k + 1]
        out += xk @ wf[k]
    mean = out.mean(0)
    var = out.var(0)
    out = (out - mean) / np.sqrt(var + EPS) * np.asarray(gamma, np.float32) \
        + np.asarray(beta, np.float32)
    return np.where(out > 0, out, LEAK * out).astype(np.float32)


def kernel(features, W, b, gamma, beta, neighbor_idx):
    from concourse.bass_utils import run_bass_kernel_spmd

    features = np.asarray(features, np.float32)
    Wf = np.asarray(W, np.float32)
    gamma_f = np.asarray(gamma, np.float32)
    beta_f = np.asarray(beta, np.float32)
    nb = np.asarray(neighbor_idx, np.int32)
    assert features.shape == (N_TOTAL, C)

    members, ok = _partition_components(nb, N_TOTAL, N_CORES, SHARD)
    if not ok:
        return _reference_fallback(features, Wf, b, gamma_f, beta_f, nb)

    plan = _make_plan(nb, members)
    core_maps = _prepare_core_inputs(features, nb, members, plan)
    w_bf = Wf.astype(ml_dtypes.bfloat16)
    for m in core_maps:
        m["w"] = w_bf
        m["gamma"] = gamma_f
        m["beta"] = beta_f

    nc = _build_bass(plan)
    res = run_bass_kernel_spmd(nc, core_maps, core_ids=list(range(N_CORES)))

    out_full = np.empty((N_TOTAL, C), np.float32)
    for c, mem in enumerate(members):
        out_full[mem] = res.results[c]["out"][:len(mem)]
    return out_full
